# revision 24
# baseline (speedup 1.0000x reference)
"""Trainium2 Bass kernel for nn_CrossAttentionFusion.

Math: softmax over kv_len==1 is identically 1.0, so the attention output is
v broadcast over the N (patch) axis and the whole module reduces to

    out[b, n, :] = cnn[b] @ (Wkv[:, C:] @ Wp) + bp        (independent of n)

W_eff = Wkv[:, C:] @ Wp is a weight-only constant, folded on the host.

Sharding: 8 cores = 4 batch-groups x 2 column-groups. Each core computes
y = cnn_shard @ W_eff_slice + bp_slice for its 16 batches x 384 columns and
writes a [16, 576, 384] output block (14.16 MB; the kernel is bound by this
HBM write stream at ~417 GB/s).

Measured machine behavior this schedule is built around:
  * a transfer's completion sem fires ~2us after its last byte (HBM
    receipt latency), and per-queue throughput scales with descriptor
    size (~150 GB/s at 1536B up to ~210+ GB/s at 6144B per queue);
  * tiny transfers at a ring head cost ~2-3us of ring latency, so the
    bias rides as a 512-col tail on a cnn half (partition 0) instead of
    its own transfer, and is accumulated LAST (K=1 chunk, bf16);
  * the PE HAM (di/dt) throttle runs matmuls at 320ns pitch (vs 162ns
    warm) and RE-ENGAGES after idle gaps, and every wait costs ~1us of
    LDW pipeline refill - so the 16 K-chunk matmuls must consume weight
    groups no faster than the sems arrive: 8 groups of 2 k-chunks
    alternate rings, giving ~1us sem cadence vs 0.64us throttled
    consumption per pair;
  * everything streams in bf16 (error ~2e-3 << 2e-2 gate); the cnn shard
    is host-replicated 8x along the lhsT M axis so the accumulation
    produces y replicated across all 128 PSUM partitions directly;
  * the replicated row is materialized 4x in SBUF (copies alternate
    Vector/Scalar engines to halve the chain) so the 9 write DMAs carry
    6144B descriptors (417 GB/s, and keeps slow SDMA engine 15 in step);
    the first two writes source from the bc4 prefix to start earlier.
"""

import sys

sys.path.insert(0, "/opt/trn_rl_repo")

import ml_dtypes
import numpy as np

import concourse.bass as bass
import concourse.mybir as mybir
from concourse import bacc
from concourse.bass_utils import run_bass_kernel_spmd
from concourse.tile import TileContext

F32 = mybir.dt.float32
BF16 = mybir.dt.bfloat16
NPBF16 = np.dtype(ml_dtypes.bfloat16)

NCORES = 8
B, N, C, CNN = 64, 576, 768, 2048
BGROUPS, CGROUPS = 4, 2          # batch groups x column groups
BS = B // BGROUPS                # 16 batches per core
CW = C // CGROUPS                # 384 columns per core
KC = CNN // 128                  # 16 k-chunks
REP = 128 // BS                  # 8 partitions per batch
ROWS_PP = N // REP               # 72 output rows per partition
RPT = 8                          # rows per partition per write DMA
NWR = ROWS_PP // RPT             # 9 write DMAs
WG = 2                           # weight k-chunks per group transfer
CTAIL = 512                      # bias cols appended to cnn_h1: ones|bp
NCOPIES = 4                      # replicated row copies in SBUF (desc size)


def _build_bass():
    nc = bacc.Bacc(None, target_bir_lowering=False, debug=False, num_devices=NCORES)

    x_cnn = nc.declare_dram_parameter(
        "cnnrep", [128, KC * 128 + CTAIL], BF16, isOutput=False
    )
    x_weff = nc.declare_dram_parameter("weff", [128, KC * CW], BF16, isOutput=False)
    y = nc.declare_dram_parameter("out", [BS, N, CW], F32, isOutput=True)

    with TileContext(nc) as tc:
        with (
            tc.tile_pool(name="singles", bufs=1) as singles,
            tc.tile_pool(name="psum_y", bufs=1, space="PSUM") as psum_y,
        ):
            # PE warm-up: junk fp32 matmul (2 passes, ~2us busy) ramps the
            # HAM di/dt throttle while the first reads stream.
            wu_sb = singles.tile([128, 512], F32, tag="wu_sb")
            nc.gpsimd.memset(wu_sb[:], 0.0)
            with tc.tile_pool(name="psum_w", bufs=1, space="PSUM") as psum_w:
                ps_w = psum_w.tile([8, 512], F32, tag="ps_w")
                nc.tensor.matmul(
                    ps_w[:], wu_sb[:, 0:8], wu_sb[:, :], start=True, stop=True
                )

            # --- read streams: cnn halves first, then 2-chunk weight
            # groups alternating rings (staggered sems, rate-matched to
            # throttled matmul consumption) ----------------------------
            half = KC * 128 // 2
            cnn_a = singles.tile([128, half], BF16, tag="cnn_a")
            cnn_b = singles.tile([128, half + CTAIL], BF16, tag="cnn_b")
            nc.sync.dma_start(out=cnn_a[:], in_=x_cnn[:, 0:half])
            nc.scalar.dma_start(out=cnn_b[:], in_=x_cnn[:, half:])

            def cnn_chunk(kc):
                t = cnn_a if kc < KC // 2 else cnn_b
                o = kc % (KC // 2)
                return t[:, o * 128 : (o + 1) * 128]

            wtiles = []
            for g in range(KC // WG):
                t = singles.tile([128, WG * CW], BF16, tag=f"wg{g}", name=f"wg{g}")
                eng = nc.sync if g % 2 == 0 else nc.scalar
                eng.dma_start(
                    out=t[:], in_=x_weff[:, g * WG * CW : (g + 1) * WG * CW]
                )
                wtiles.append(t)

            # --- compute: y replicated across 128 partitions -----------
            ps_y = psum_y.tile([128, CW], F32, tag="ps_y")
            for kc in range(KC):
                nc.tensor.matmul(
                    ps_y[:],
                    cnn_chunk(kc),
                    wtiles[kc // WG][:, (kc % WG) * CW : (kc % WG + 1) * CW],
                    start=(kc == 0),
                    stop=False,
                )
            # bias last: ps_y[p, c] += 1 * bp[c]; ones|bp ride cnn_b's
            # tail on partition 0
            nc.tensor.matmul(
                ps_y[:],
                cnn_b[0:1, half : half + 128],
                cnn_b[0:1, half + 128 : half + 128 + CW],
                start=False,
                stop=True,
            )

            # materialize NCOPIES of the row; alternate Vector/Scalar
            # engines so the copy chain is ~2x shorter
            bc4 = singles.tile([128, NCOPIES * CW], F32, tag="bc4")
            for j in range(NCOPIES):
                if j % 2 == 0:
                    nc.vector.tensor_copy(bc4[:, j * CW : (j + 1) * CW], ps_y[:])
                else:
                    nc.scalar.copy(bc4[:, j * CW : (j + 1) * CW], ps_y[:])

            # out rows n = q*72 + s for partition p = b*8 + q; each DMA
            # writes RPT consecutive rows per partition. Sources grow with
            # the bc4 prefix so early writes launch before all copies land.
            y_v = y.rearrange("b (q s) c -> (b q) s c", q=REP)
            srcs = {
                0: bc4[:, 0:CW].unsqueeze(1).broadcast_to((128, RPT, CW)),
                1: bc4[:, 0 : 2 * CW]
                .unsqueeze(1)
                .broadcast_to((128, RPT // 2, 2 * CW)),
            }
            src_full = (
                bc4[:, :]
                .unsqueeze(1)
                .broadcast_to((128, RPT // NCOPIES, NCOPIES * CW))
            )
            for i in range(NWR):
                eng = nc.sync if i % 2 == 0 else nc.scalar
                eng.dma_start(
                    out=y_v[:, i * RPT : (i + 1) * RPT, :],
                    in_=srcs.get(i, src_full),
                )

    nc.compile()
    return nc


_NC = None


def _get_nc():
    global _NC
    if _NC is None:
        _NC = _build_bass()
    return _NC


def _prepare_in_maps(image_patches, cnn_feature_vector, Wq, Wkv, Wp, bp):
    Weff = np.ascontiguousarray(Wkv[:, C:]) @ Wp  # (2048, 768) fp32
    bp = bp.astype(np.float32)

    weff_arrs = []
    for cg in range(CGROUPS):
        sl = slice(cg * CW, (cg + 1) * CW)
        weff_arrs.append(
            np.ascontiguousarray(
                Weff[:, sl]
                .reshape(KC, 128, CW)
                .transpose(1, 0, 2)
                .reshape(128, KC * CW)
                .astype(NPBF16)
            )
        )

    cnn_arrs = []
    for bg in range(BGROUPS):
        shard = cnn_feature_vector[bg * BS : (bg + 1) * BS]  # (16, 2048)
        rep = np.repeat(shard, REP, axis=0)  # (128, 2048), row p = batch p//8
        arr = np.zeros((128, KC * 128 + CTAIL), dtype=NPBF16)
        arr[:, : KC * 128] = (
            rep.reshape(128, KC, 128).transpose(2, 1, 0).reshape(128, KC * 128)
        ).astype(NPBF16)
        # bias tail on partition 0: ones(128) | bp-slice placeholder
        arr[0, KC * 128 : KC * 128 + 128] = np.float32(1.0)
        cnn_arrs.append(arr)

    in_maps = []
    for core in range(NCORES):
        bg, cg = core // CGROUPS, core % CGROUPS
        arr = cnn_arrs[bg].copy()
        arr[0, KC * 128 + 128 : KC * 128 + 128 + CW] = bp[
            cg * CW : (cg + 1) * CW
        ].astype(NPBF16)
        in_maps.append({"cnnrep": arr, "weff": weff_arrs[cg]})
    return in_maps


def _assemble(res):
    out = np.empty((B, N, C), dtype=np.float32)
    for core in range(NCORES):
        bg, cg = core // CGROUPS, core % CGROUPS
        out[bg * BS : (bg + 1) * BS, :, cg * CW : (cg + 1) * CW] = res.results[
            core
        ]["out"]
    return out


def kernel(**inputs) -> np.ndarray:
    inputs = {k: np.asarray(v) for k, v in inputs.items()}
    nc = _get_nc()
    in_maps = _prepare_in_maps(**inputs)
    res = run_bass_kernel_spmd(nc, in_maps, core_ids=list(range(NCORES)))
    return _assemble(res)


def kernel_traced(**inputs):
    """kernel() + HW profile; returns (output, BassKernelResults)."""
    inputs = {k: np.asarray(v) for k, v in inputs.items()}
    nc = _get_nc()
    in_maps = _prepare_in_maps(**inputs)
    res = run_bass_kernel_spmd(
        nc, in_maps, core_ids=list(range(NCORES)), trace=True
    )
    return _assemble(res), res
